# revision 1
# baseline (speedup 1.0000x reference)
"""Trainium2 Bass kernel for nn_CrossAttention (16x512x64x64, 8 heads x 64).

Math notes (exact algebraic restructuring of the reference):
  The reference tiles ky=[b,1,1,c] to k=[b,c,1,c] before conv1x1(to_k_w), so
  every input channel of that conv carries the same value ky[b,j].  Hence
    conv1x1(k, to_k_w)[b,o,0,j] = rowsum(to_k_w)[o] * ky[b,j]     (rank-1)
  and likewise for v with rowsum(to_v_w) and vy.  Propagating this:
    ksm[b,hd,j] = softmax_j(rs_k[hd] * ky[b,j])
    w[b,hd]     = sum_j ksm[b,hd,j] * vy[b,j]
    context[b,h,d,e] = w[b,h,d] * rs_v[h,e]                        (rank-1)
    out[b,he,n] = rs_v[he] * s[b,h,n],  s = sum_d softmax_d(q)[d,n] * w[h,d]
    final[b,o,n] = sum_h W2[o,h] * s[b,h,n] + out_b[o],
      with W2[o,h] = scale * sum_e out_w[o, h*64+e] * rs_v[h*64+e]
  followed by GroupNorm(1) over (C,H,W) per sample.

  The only large compute left is q = to_q_w @ x (2.1 GFLOP/sample), computed
  transposed (qT[n,he] = x[c,n]^T @ to_q_wT[c,he]) so the d-softmax is a
  free-dim reduction over 64-wide head chunks.

Sharding: data-parallel over batch, 2 samples per core, 8 cores, no
collectives.  Each core gets the full weights.
"""

import numpy as np

import concourse.bass as bass
import concourse.mybir as mybir
import concourse.tile as tile
from concourse import bacc
from concourse.bass import ts
from concourse.bass_utils import run_bass_kernel_spmd

B, C, N = 16, 512, 4096
DIMY = 768
HEADS, DHEAD = 8, 64
NCORES = 8
BPC = B // NCORES  # samples per core
SCALE = DHEAD ** -0.5
EPS = 1e-5
F32 = mybir.dt.float32
F32R = mybir.dt.float32r
BF16 = mybir.dt.bfloat16
AX = mybir.AxisListType.X
AF = mybir.ActivationFunctionType
OP = mybir.AluOpType


def build_nc(use_f32r=True):
    MDT = F32R if use_f32r else F32

    nc = bacc.Bacc()
    xd = nc.dram_tensor("x", [BPC, C, N], F32, kind="ExternalInput")
    yd = nc.dram_tensor("y", [BPC, DIMY], F32, kind="ExternalInput")
    kwd = nc.dram_tensor("k_w", [C, DIMY], F32, kind="ExternalInput")
    vwd = nc.dram_tensor("v_w", [C, DIMY], F32, kind="ExternalInput")
    qwd = nc.dram_tensor("to_q_w", [C, C], F32, kind="ExternalInput")
    tkd = nc.dram_tensor("to_k_w", [C, C], F32, kind="ExternalInput")
    tvd = nc.dram_tensor("to_v_w", [C, C], F32, kind="ExternalInput")
    owd = nc.dram_tensor("out_w", [C, C], F32, kind="ExternalInput")
    obd = nc.dram_tensor("out_b", [C], F32, kind="ExternalInput")
    gngd = nc.dram_tensor("gn_g", [C], F32, kind="ExternalInput")
    gnbd = nc.dram_tensor("gn_b", [C], F32, kind="ExternalInput")
    outd = nc.dram_tensor("out", [BPC, C, N], F32, kind="ExternalOutput")

    from contextlib import ExitStack

    with tile.TileContext(nc) as tc, ExitStack() as ctx:
        persist = ctx.enter_context(tc.tile_pool(name="persist", bufs=1))
        prep = ctx.enter_context(tc.tile_pool(name="prep", bufs=1))
        bcastp = ctx.enter_context(tc.tile_pool(name="bcast", bufs=5))
        ezp = ctx.enter_context(tc.tile_pool(name="ezp", bufs=2))
        eqp = ctx.enter_context(tc.tile_pool(name="eqp", bufs=3))
        workp = ctx.enter_context(tc.tile_pool(name="workp", bufs=3))
        xp = ctx.enter_context(tc.tile_pool(name="xp", bufs=10))
        sttp = ctx.enter_context(tc.tile_pool(name="sttp", bufs=18))
        stgp = ctx.enter_context(tc.tile_pool(name="stgp", bufs=6))
        tep = ctx.enter_context(tc.tile_pool(name="tep", bufs=6))
        smallp = ctx.enter_context(tc.tile_pool(name="smallp", bufs=6))
        rowp = ctx.enter_context(tc.tile_pool(name="rowp", bufs=2))
        statsp = ctx.enter_context(tc.tile_pool(name="statsp", bufs=2))
        ybcp = ctx.enter_context(tc.tile_pool(name="ybcp", bufs=1))
        psqp = ctx.enter_context(tc.tile_pool(name="psqp", bufs=3, space="PSUM"))
        psfp = ctx.enter_context(tc.tile_pool(name="psfp", bufs=3, space="PSUM"))
        psf2p = psfp
        psmp = ctx.enter_context(tc.tile_pool(name="psmp", bufs=2, space="PSUM"))

        def bcast_row(src_row_ap, n, tag, dt=F32):
            """Broadcast a [1, n] SBUF row to [128, n] via a K=1 PE matmul
            against a ones row (internal-DRAM scratch fails to load here)."""
            ps_b = psmp.tile([128, n], F32, tag="pm")
            nc.tensor.matmul(ps_b, lhsT=ones_row, rhs=src_row_ap, start=True, stop=True)
            b = bcastp.tile([128, n], dt, tag="bc" if n == C else "bc_" + tag)
            nc.scalar.copy(out=b, in_=ps_b)
            return b

        # ---------------- prep (sample independent) ----------------
        ident = persist.tile([128, 128], F32, tag="ident")
        from concourse.masks import make_identity

        make_identity(nc, ident)
        ones_col = persist.tile([128, 1], F32, tag="ones")
        nc.vector.memset(ones_col, 1.0)
        ones_row = persist.tile([1, 128], F32, tag="onesr")
        nc.vector.memset(ones_row, 1.0)
        zero_col = persist.tile([128, 1], F32, tag="zero")
        nc.vector.memset(zero_col, 0.0)
        nc.const_aps.aps[(F32, 0.0)] = zero_col[:, :]
        eps_col = persist.tile([128, 1], F32, tag="eps")
        nc.vector.memset(eps_col, EPS)
        nc.const_aps.aps[(F32, EPS)] = eps_col[:, :]

        # per-o columns [128, 4]: col i holds values for o in [i*128,(i+1)*128)
        outb_col = persist.tile([128, 4], F32, tag="outb")
        nc.sync.dma_start(out=outb_col, in_=obd.rearrange("(i p) -> p i", p=128))
        gng_col = persist.tile([128, 4], F32, tag="gng")
        nc.sync.dma_start(out=gng_col, in_=gngd.rearrange("(i p) -> p i", p=128))
        gnb_col = persist.tile([128, 4], F32, tag="gnb")
        nc.sync.dma_start(out=gnb_col, in_=gnbd.rearrange("(i p) -> p i", p=128))

        # to_q_w transposed -> qwT[:, ct, :] = to_q_w.T[ct*128:(ct+1)*128, :]
        tq_nat = prep.tile([128, 4, DIMY], F32, tag="wnat")
        nc.sync.dma_start(
            out=tq_nat[:, :, :C], in_=qwd.rearrange("(i p) c -> p i c", p=128)
        )
        qwT = persist.tile([128, 4, C], BF16, tag="qwT")
        for ct in range(4):
            for ot in range(4):
                pst = psmp.tile([128, 128], F32, tag="pm")
                nc.tensor.transpose(pst, tq_nat[:, ot, ts(ct, 128)], ident)
                nc.scalar.copy(out=qwT[:, ct, ts(ot, 128)], in_=pst)

        # row sums of to_k_w / to_v_w  -> [128, 4] columns
        rsk_col = persist.tile([128, 4], F32, tag="rsk")
        rsv_col = persist.tile([128, 4], F32, tag="rsv")
        for dram, col in ((tkd, rsk_col), (tvd, rsv_col)):
            nat = prep.tile([128, 4, DIMY], F32, tag="wnat")
            nc.sync.dma_start(
                out=nat[:, :, :C], in_=dram.rearrange("(i p) c -> p i c", p=128)
            )
            for ot in range(4):
                nc.vector.reduce_sum(
                    out=col[:, ot : ot + 1], in_=nat[:, ot, :C], axis=AX
                )

        # rs_v as a broadcast row, scaled by softmax scale (folded into W2)
        ps_row = psmp.tile([1, C], F32, tag="pm")
        for ot in range(4):
            nc.tensor.transpose(
                ps_row[:, ts(ot, 128)], rsv_col[:, ot : ot + 1], ident
            )
        rsv_row = rowp.tile([1, C], F32, tag="rsvrow")
        nc.scalar.mul(out=rsv_row, in_=ps_row, mul=SCALE)
        rsv_b = bcast_row(rsv_row, C, "rsv")

        # W2T[h, ot, :]: W2[o,h] = sum_e out_w[o, h*64+e] * rs_v[h*64+e] * scale
        ow_nat = prep.tile([128, 4, DIMY], F32, tag="wnat")
        nc.sync.dma_start(
            out=ow_nat[:, :, :C], in_=owd.rearrange("(i p) c -> p i c", p=128)
        )
        w2T = persist.tile([HEADS, 4, 128], MDT, tag="w2T")
        for ot in range(4):
            t_ = workp.tile([128, C], F32, tag="tmp")
            nc.vector.tensor_mul(t_, ow_nat[:, ot, :C], rsv_b)
            w2c = smallp.tile([128, HEADS], F32, tag="w2c")
            nc.vector.reduce_sum(
                out=w2c, in_=t_.rearrange("p (h d) -> p h d", d=DHEAD), axis=AX
            )
            psw = psmp.tile([HEADS, 128], F32, tag="pm")
            nc.tensor.transpose(psw, w2c, ident)
            nc.scalar.copy(out=w2T[:, ot, :], in_=psw)

        # ky / vy rows per sample: ky[b,o] = sum_d y[b,d] * k_w[o,d]
        kyvy = persist.tile([1, 2 * BPC, C], F32, tag="kyvy")  # [kv*BPC+s]
        for kv, dram in ((0, kwd), (1, vwd)):
            nat = prep.tile([128, 4, DIMY], F32, tag="kvnat")
            nc.sync.dma_start(out=nat, in_=dram.rearrange("(i p) d -> p i d", p=128))
            for s in range(BPC):
                y_b = ybcp.tile([128, DIMY], F32, tag="yb")
                nc.gpsimd.dma_start(out=y_b, in_=yd[s].partition_broadcast(128))
                col = smallp.tile([128, 4], F32, tag="kycol")
                for ot in range(4):
                    scr = ybcp.tile([128, DIMY], F32, tag="yscr")
                    nc.vector.tensor_mul(scr, nat[:, ot, :], y_b)
                    nc.vector.reduce_sum(
                        out=col[:, ot : ot + 1], in_=scr, axis=AX
                    )
                psr = psmp.tile([1, C], F32, tag="pm")
                for ot in range(4):
                    nc.tensor.transpose(
                        psr[:, ts(ot, 128)], col[:, ot : ot + 1], ident
                    )
                nc.scalar.copy(out=kyvy[:, kv * BPC + s, :], in_=psr)

        # ---------------- per-sample main ----------------
        for s in range(BPC):
            ky_b = bcast_row(kyvy[:, s, :], C, "ky")
            vy_b = bcast_row(kyvy[:, BPC + s, :], C, "vy")

            # k-softmax + weighting: w[hd] = sum_j softmax_j(rs_k[hd]*ky[j]) vy[j]
            den_k = smallp.tile([128, 4], F32, tag="denk")
            num_k = smallp.tile([128, 4], F32, tag="numk")
            for t in range(4):
                ez = ezp.tile([128, C], F32, tag="ez")
                nc.scalar.activation(
                    out=ez,
                    in_=ky_b,
                    func=AF.Exp,
                    scale=rsk_col[:, t : t + 1],
                )
                nc.vector.reduce_sum(
                    out=den_k[:, t : t + 1], in_=ez, axis=AX
                )
                scr = workp.tile([128, C], F32, tag="tmp")
                nc.vector.tensor_mul(scr, ez, vy_b)
                nc.vector.reduce_sum(
                    out=num_k[:, t : t + 1], in_=scr, axis=AX
                )
            denr_k = smallp.tile([128, 4], F32, tag="denrk")
            nc.vector.reciprocal(denr_k, den_k)
            w_col = smallp.tile([128, 4], F32, tag="wcol")
            nc.vector.tensor_mul(w_col, num_k, denr_k)
            ps_w = psmp.tile([1, C], F32, tag="pm")
            for t in range(4):
                nc.tensor.transpose(ps_w[:, ts(t, 128)], w_col[:, t : t + 1], ident)
            w_row = rowp.tile([1, C], F32, tag="wrow")
            nc.scalar.copy(out=w_row, in_=ps_w)
            w_b = bcast_row(w_row, C, "w", dt=BF16)

            stats = statsp.tile([128, 4, 8, 6], F32, tag="stats")
            stt_tiles = []
            for g in range(8):  # n-groups of 512
                xcs = []
                for ct in range(4):
                    xc = xp.tile([128, 512], BF16, tag="xc")
                    nc.gpsimd.dma_start(
                        out=xc,
                        in_=xd[s, ts(ct, 128), ts(g, 512)],
                    )
                    xcs.append(xc)
                ps_stt = psmp.tile([HEADS, 512], F32, tag="pm")
                for j in range(4):  # n-tiles of 128 within the group
                    psq = psqp.tile([128, 512], F32, tag="psq")
                    for ct in range(4):
                        nc.tensor.matmul(
                            psq,
                            lhsT=xcs[ct][:, ts(j, 128)],
                            rhs=qwT[:, ct, :],
                            start=(ct == 0),
                            stop=(ct == 3),
                        )
                    te = tep.tile([128, 2, 512], BF16, tag="te")
                    nc.scalar.activation(out=te[:, 1, :], in_=psq, func=AF.Exp)
                    nc.gpsimd.tensor_mul(te[:, 0, :], te[:, 1, :], w_b)
                    sn2 = smallp.tile([128, 2, HEADS], F32, tag="sn2")
                    nc.vector.reduce_sum(
                        out=sn2,
                        in_=te.rearrange("p t (h d) -> p t h d", d=DHEAD),
                        axis=AX,
                    )
                    s_denr = smallp.tile([128, HEADS], F32, tag="sdenr")
                    nc.vector.reciprocal(s_denr, sn2[:, 1, :])
                    s_t = smallp.tile([128, HEADS], F32, tag="stile")
                    nc.vector.tensor_mul(s_t, sn2[:, 0, :], s_denr)
                    nc.tensor.transpose(ps_stt[:, ts(j, 128)], s_t, ident)
                stt = sttp.tile([HEADS, 512], MDT, tag="stt")
                nc.scalar.copy(out=stt, in_=ps_stt)
                stt_tiles.append(stt)
                for ot in range(4):
                    psf = psfp.tile([128, 512], F32, tag="psf")
                    nc.tensor.matmul(
                        psf,
                        lhsT=w2T[:, ot, :],
                        rhs=stt,
                        start=True,
                        stop=True,
                    )
                    nc.vector.bn_stats(out=stats[:, ot, g, :], in_=psf)

            # ---- GroupNorm(1) stats over the whole sample ----
            mvacc = smallp.tile([128, 2, 4], F32, tag="mvacc")
            for ot in range(4):
                mv = smallp.tile([128, 2], F32, tag="mv")
                nc.vector.bn_aggr(out=mv, in_=stats[:, ot, :, :])
                m_ = mvacc[:, 0, ot : ot + 1]
                nc.vector.tensor_add(m_, mv[:, 0:1], outb_col[:, ot : ot + 1])
                msq = smallp.tile([128, 1], F32, tag="msq")
                nc.vector.tensor_mul(msq, m_, m_)
                nc.vector.tensor_add(mvacc[:, 1, ot : ot + 1], mv[:, 1:2], msq)
            mv_tot = smallp.tile([128, 2], F32, tag="mvtot")
            nc.vector.reduce_sum(out=mv_tot, in_=mvacc, axis=AX)
            ps_tot = psmp.tile([1, 2], F32, tag="pm")
            nc.tensor.matmul(ps_tot, lhsT=ones_col, rhs=mv_tot, start=True, stop=True)
            tt = rowp.tile([1, 4], F32, tag="tt")
            nc.scalar.mul(out=tt[:, 0:2], in_=ps_tot, mul=1.0 / C)
            nc.vector.tensor_mul(tt[:, 2:3], tt[:, 0:1], tt[:, 0:1])  # mu^2
            nc.vector.tensor_sub(tt[:, 3:4], tt[:, 1:2], tt[:, 2:3])  # var
            sd = rowp.tile([1, 1], F32, tag="sd")
            nc.scalar.activation(out=sd, in_=tt[:, 3:4], func=AF.Sqrt, bias=EPS)
            rstd = rowp.tile([1, 1], F32, tag="rstd")
            nc.vector.reciprocal(rstd, sd)
            murow = rowp.tile([1, 2], F32, tag="mur")
            nc.vector.tensor_copy(murow[:, 0:1], tt[:, 0:1])
            nc.vector.tensor_copy(murow[:, 1:2], rstd)
            ms_b = bcast_row(murow, 2, "ms")

            # A = gn_g * rstd ; B = A*(out_b - mu) + gn_b ; out = A*mm + B
            a_col = smallp.tile([128, 4], F32, tag="acol")
            nc.vector.tensor_scalar_mul(a_col, gng_col, ms_b[:, 1:2])
            t1 = smallp.tile([128, 4], F32, tag="t1")
            nc.vector.tensor_scalar(
                out=t1, in0=outb_col, scalar1=ms_b[:, 0:1], scalar2=None,
                op0=OP.subtract,
            )
            t2 = smallp.tile([128, 4], F32, tag="t2")
            nc.vector.tensor_mul(t2, a_col, t1)
            b_col = smallp.tile([128, 4], F32, tag="bcol")
            nc.vector.tensor_add(b_col, t2, gnb_col)

            # rows: A and B2 as [1, 512] rows, A broadcast to 8 partitions
            ps_a = psmp.tile([1, C], F32, tag="pm")
            for ot in range(4):
                nc.tensor.transpose(
                    ps_a[:, ts(ot, 128)], a_col[:, ot : ot + 1], ident
                )
            a_row = rowp.tile([1, C], F32, tag="arow")
            nc.scalar.copy(out=a_row, in_=ps_a)
            ps_a8 = psmp.tile([HEADS, C], F32, tag="pm")
            nc.tensor.matmul(
                ps_a8,
                lhsT=ones_row[:, 0:HEADS],
                rhs=a_row,
                start=True,
                stop=True,
            )
            a8_sb = rowp.tile([HEADS, C], F32, tag="a8")
            nc.scalar.copy(out=a8_sb, in_=ps_a8)
            # w2s = W2T * A(o); B2(o) is added as bias in the staging copy
            w2s = rowp.tile([HEADS, 4, 128], MDT, tag="w2s")
            nc.vector.tensor_mul(
                w2s,
                w2T,
                a8_sb.rearrange("p (i f) -> p i f", i=4),
            )
            for g in range(8):
                for ot in range(4):
                    psf2 = psf2p.tile([128, 512], F32, tag="psf")
                    nc.tensor.matmul(
                        psf2,
                        lhsT=w2s[:, ot, :],
                        rhs=stt_tiles[g],
                        start=True,
                        stop=True,
                    )
                    stg = stgp.tile([128, 512], F32, tag="stg")
                    nc.scalar.activation(
                        out=stg,
                        in_=psf2,
                        func=AF.Identity,
                        bias=b_col[:, ot : ot + 1],
                    )
                    nc.sync.dma_start(
                        out=outd[s, ts(ot, 128), ts(g, 512)], in_=stg
                    )

    nc.finalize()
    return nc


_NC_CACHE = {}


def _get_nc(use_f32r=True):
    if use_f32r not in _NC_CACHE:
        _NC_CACHE[use_f32r] = build_nc(use_f32r)
    return _NC_CACHE[use_f32r]


def make_in_maps(inputs):
    x = np.ascontiguousarray(inputs["x"], dtype=np.float32).reshape(B, C, N)
    y = np.ascontiguousarray(inputs["y"], dtype=np.float32).reshape(B, DIMY)
    shared = {
        k: np.ascontiguousarray(inputs[k], dtype=np.float32)
        for k in (
            "k_w", "v_w", "to_q_w", "to_k_w", "to_v_w", "out_w",
            "out_b", "gn_g", "gn_b",
        )
    }
    in_maps = []
    for core in range(NCORES):
        s0 = core * BPC
        m = {"x": x[s0 : s0 + BPC], "y": y[s0 : s0 + BPC]}
        m.update(shared)
        in_maps.append(m)
    return in_maps


def kernel(**inputs):
    nc = _get_nc(use_f32r=True)
    res = run_bass_kernel_spmd(nc, make_in_maps(inputs), list(range(NCORES)))
    out = np.concatenate([r["out"] for r in res.results], axis=0)
    return out.reshape(B, C, 64, 64)


if __name__ == "__main__":
    rng = np.random.default_rng(0)
    inputs = {
        "x": rng.standard_normal((B, C, 64, 64), dtype=np.float32),
        "y": rng.standard_normal((B, 1, 1, DIMY), dtype=np.float32),
        "k_w": rng.standard_normal((C, DIMY), dtype=np.float32) * 0.02,
        "v_w": rng.standard_normal((C, DIMY), dtype=np.float32) * 0.02,
        "to_q_w": rng.standard_normal((C, C), dtype=np.float32) * 0.02,
        "to_k_w": rng.standard_normal((C, C), dtype=np.float32) * 0.02,
        "to_v_w": rng.standard_normal((C, C), dtype=np.float32) * 0.02,
        "out_w": rng.standard_normal((C, C), dtype=np.float32) * 0.02,
        "out_b": np.zeros(C, np.float32),
        "gn_g": np.ones(C, np.float32),
        "gn_b": np.zeros(C, np.float32),
    }
    out = kernel(**inputs)
    print("kernel ran, out shape", out.shape, "std", out.std())



# revision 10
# speedup vs baseline: 1.5916x; 1.5916x over previous
"""Trainium2 Bass kernel for nn_CrossAttention (16x512x64x64, 8 heads x 64).

Math (exact algebraic restructuring of the reference; see baseline notes):
  The tiled-k/v convs are rank-1, so everything except q = to_q_w @ x
  collapses to per-sample vectors computed on HOST from weights+y:
    w[b,hd]  = sum_j softmax_j(rs_k[hd]*ky[b,j]) * vy[b,j]
    W2[o,h]  = scale * sum_e out_w[o,64h+e] * rs_v[64h+e]
  Device computes, per sample:
    q[he,n]   = to_q_w @ x              (PE, bf16, [he,n] layout)
    e         = exp(q)                  (ACT)
    num/den   = mask-matmul over he     (PE: lhsT = [w-masked | 1-masked])
    s[h,n]    = num/den                 (DVE)
    mm[o,n]   = W2 @ s                  (PE, K=8)
    stats     = sum/sumsq of mm rows    (accum_out on the PSUM->SBUF copy + DVE)
    out       = A(o)*mm + B(o)          (GroupNorm affine, GPSIMD, in-place)

Sharding: data-parallel over batch, 2 samples/core, 8 cores, no collectives.
All I/O in bf16 (host converts); weights pre-transposed on host.
"""

import numpy as np
import ml_dtypes

import concourse.bass as bass
import concourse.mybir as mybir
import concourse.tile as tile
from concourse import bacc
from concourse.bass import ts
from concourse.bass_utils import run_bass_kernel_spmd

B, C, N = 16, 512, 4096
DIMY = 768
HEADS, DHEAD = 8, 64
NCORES = 8
BPC = B // NCORES
SCALE = DHEAD ** -0.5
EPS = 1e-5
F32 = mybir.dt.float32
BF16 = mybir.dt.bfloat16
AX = mybir.AxisListType.X
AF = mybir.ActivationFunctionType
OP = mybir.AluOpType
BF16NP = ml_dtypes.bfloat16


def build_nc(use_f32r=True):
    nc = bacc.Bacc()
    xd = nc.dram_tensor("xb", [BPC, 4, 128, N], BF16, kind="ExternalInput")
    qwtd = nc.dram_tensor("qwt", [128, 4, C], BF16, kind="ExternalInput")
    wmd = nc.dram_tensor("wm", [128, BPC, 4, 40], BF16, kind="ExternalInput")
    w2td = nc.dram_tensor("w2t", [HEADS, 4, 128], BF16, kind="ExternalInput")
    gcd = nc.dram_tensor("gcols", [128, 4, 3], F32, kind="ExternalInput")
    outd = nc.dram_tensor("out", [BPC, 4, 128, N], BF16, kind="ExternalOutput")

    from contextlib import ExitStack

    with tile.TileContext(nc) as tc, ExitStack() as ctx:
        persist = ctx.enter_context(tc.tile_pool(name="persist", bufs=1))
        xp = ctx.enter_context(tc.tile_pool(name="xp", bufs=8))
        ep = ctx.enter_context(tc.tile_pool(name="ep", bufs=6))
        sttp = ctx.enter_context(tc.tile_pool(name="sttp", bufs=4))
        rdp = ctx.enter_context(tc.tile_pool(name="rdp", bufs=3))
        mmp = ctx.enter_context(tc.tile_pool(name="mmp", bufs=2))
        accp = ctx.enter_context(tc.tile_pool(name="accp", bufs=4))
        scrp = ctx.enter_context(tc.tile_pool(name="scrp", bufs=2))
        smallp = ctx.enter_context(tc.tile_pool(name="smallp", bufs=10))
        rowp = ctx.enter_context(tc.tile_pool(name="rowp", bufs=8))
        psqp = ctx.enter_context(tc.tile_pool(name="psqp", bufs=3, space="PSUM"))
        ps16p = ctx.enter_context(tc.tile_pool(name="ps16p", bufs=2, space="PSUM"))
        psfp = ctx.enter_context(tc.tile_pool(name="psfp", bufs=3, space="PSUM"))

        # ---------------- weights / constants ----------------
        qwt = persist.tile([128, 4, C], BF16, tag="qwt")
        nc.sync.dma_start(out=qwt, in_=qwtd[:, :, :])
        wm = persist.tile([128, BPC, 4, 40], BF16, tag="wm")
        nc.sync.dma_start(out=wm, in_=wmd[:, :, :, :])
        w2t = persist.tile([HEADS, 4, 128], BF16, tag="w2t")
        nc.sync.dma_start(out=w2t, in_=w2td[:, :, :])
        gcols = persist.tile([128, 4, 3], F32, tag="gcols")
        nc.sync.dma_start(out=gcols, in_=gcd[:, :, :])
        outb_c = gcols[:, :, 0]
        gng_c = gcols[:, :, 1]
        gnb_c = gcols[:, :, 2]

        ones_col = persist.tile([128, 1], F32, tag="ones")
        nc.vector.memset(ones_col, 1.0)
        ones_row = persist.tile([1, 128], F32, tag="onesr")
        nc.vector.memset(ones_row, 1.0)
        zero_col = persist.tile([128, 1], F32, tag="zero")
        nc.vector.memset(zero_col, 0.0)
        nc.const_aps.aps[(F32, 0.0)] = zero_col[:, :]
        eps_col = persist.tile([128, 1], F32, tag="eps")
        nc.vector.memset(eps_col, EPS)
        nc.const_aps.aps[(F32, EPS)] = eps_col[:, :]

        # all x tiles up-front (sync HWDGE queue, 1 MB each)
        xs = {}
        for s in range(BPC):
            for ct in range(4):
                t = xp.tile([128, N], BF16, tag="xc")
                nc.sync.dma_start(out=t, in_=xd[s, ct, :, :])
                xs[(s, ct)] = t

        for s in range(BPC):
            mmb = mmp.tile([128, 4, N], BF16, tag="mmb")
            s1t = accp.tile([128, 4, 8], F32, tag="s1t")
            s2t = accp.tile([128, 4, 8], F32, tag="s2t")

            for g in range(8):
                es = []
                for ot in range(4):
                    psq = psqp.tile([128, 512], F32, tag="psq")
                    for ct in range(4):
                        nc.tensor.matmul(
                            psq,
                            lhsT=qwt[:, ct, ts(ot, 128)],
                            rhs=xs[(s, ct)][:, ts(g, 512)],
                            start=(ct == 0),
                            stop=(ct == 3),
                        )
                    e = ep.tile([128, 512], BF16, tag="e")
                    nc.scalar.activation(out=e, in_=psq, func=AF.Exp)
                    es.append(e)

                ps16 = ps16p.tile([40, 512], F32, tag="ps16")
                for ot in range(4):
                    nc.tensor.matmul(
                        ps16,
                        lhsT=wm[:, s, ot, :],
                        rhs=es[ot],
                        start=(ot == 0),
                        stop=(ot == 3),
                    )
                rden = rdp.tile([8, 512], F32, tag="rden")
                nc.vector.reciprocal(rden, ps16[32:40, :])
                stt = sttp.tile([8, 512], BF16, tag="stt")
                nc.vector.tensor_mul(stt, ps16[0:8, :], rden)

                for ot in range(4):
                    psf = psfp.tile([128, 512], F32, tag="psf")
                    nc.tensor.matmul(
                        psf,
                        lhsT=w2t[:, ot, :],
                        rhs=stt,
                        start=True,
                        stop=True,
                    )
                    dst = mmb[:, ot, ts(g, 512)]
                    nc.vector.tensor_scalar(
                        out=dst, in0=psf, scalar1=1.0, scalar2=0.0,
                        op0=OP.mult, op1=OP.add,
                        accum_out=s1t[:, ot, g : g + 1],
                    )
                    mm2 = scrp.tile([128, 512], BF16, tag="mm2")
                    nc.scalar.activation(
                        out=mm2, in_=psf, func=AF.Square,
                        accum_out=s2t[:, ot, g : g + 1],
                    )

            # ---- GroupNorm(1) stats + affine ----
            s1tot = smallp.tile([128, 4], F32, tag="s1tot")
            nc.vector.reduce_sum(out=s1tot, in_=s1t, axis=AX)
            s2tot = smallp.tile([128, 4], F32, tag="s2tot")
            nc.vector.reduce_sum(out=s2tot, in_=s2t, axis=AX)
            mv8 = smallp.tile([128, 8], F32, tag="mv8")
            mraw = smallp.tile([128, 4], F32, tag="mraw")
            nc.vector.tensor_scalar_mul(mraw, s1tot, 1.0 / N)
            nc.vector.tensor_add(mv8[:, 0:4], mraw, outb_c)  # mrow
            e2a = smallp.tile([128, 4], F32, tag="e2a")
            nc.vector.tensor_scalar_mul(e2a, s2tot, 1.0 / N)
            tmp = smallp.tile([128, 4], F32, tag="tmp")
            nc.vector.tensor_add(tmp, mv8[:, 0:4], mraw)
            tmp2 = smallp.tile([128, 4], F32, tag="tmp2")
            nc.vector.tensor_mul(tmp2, tmp, outb_c)
            nc.vector.tensor_add(mv8[:, 4:8], e2a, tmp2)  # e2row

            ps_tot = ps16p.tile([1, 8], F32, tag="ps16")
            nc.tensor.matmul(ps_tot, lhsT=ones_col, rhs=mv8, start=True, stop=True)
            rowt8 = rowp.tile([1, 8], F32, tag="rowt8")
            nc.scalar.copy(out=rowt8, in_=ps_tot)
            tt = rowp.tile([1, 2], F32, tag="tt")
            nc.vector.reduce_sum(
                out=tt, in_=rowt8.rearrange("p (a b) -> p a b", a=2), axis=AX
            )
            tt2 = rowp.tile([1, 2], F32, tag="tt2")
            nc.scalar.mul(out=tt2, in_=tt, mul=1.0 / C)  # {mu, E2}
            msq = rowp.tile([1, 1], F32, tag="msq")
            nc.vector.tensor_mul(msq, tt2[:, 0:1], tt2[:, 0:1])
            var = rowp.tile([1, 1], F32, tag="var")
            nc.vector.tensor_sub(var, tt2[:, 1:2], msq)
            sd = rowp.tile([1, 1], F32, tag="sd")
            nc.scalar.activation(out=sd, in_=var, func=AF.Sqrt, bias=EPS)
            rstd = rowp.tile([1, 1], F32, tag="rstd")
            nc.vector.reciprocal(rstd, sd)
            murow = rowp.tile([1, 2], F32, tag="murow")
            nc.vector.tensor_copy(murow[:, 0:1], tt2[:, 0:1])
            nc.vector.tensor_copy(murow[:, 1:2], rstd)
            ps_b = ps16p.tile([128, 2], F32, tag="ps16")
            nc.tensor.matmul(ps_b, lhsT=ones_row, rhs=murow, start=True, stop=True)
            msb = smallp.tile([128, 2], F32, tag="msb")
            nc.scalar.copy(out=msb, in_=ps_b)

            a_col = smallp.tile([128, 4], F32, tag="acol")
            nc.vector.tensor_scalar_mul(a_col, gng_c, msb[:, 1:2])
            t1 = smallp.tile([128, 4], F32, tag="t1")
            nc.vector.tensor_scalar(
                out=t1, in0=outb_c, scalar1=msb[:, 0:1], scalar2=None,
                op0=OP.subtract,
            )
            t2 = smallp.tile([128, 4], F32, tag="t2")
            nc.vector.tensor_mul(t2, a_col, t1)
            b2 = smallp.tile([128, 4], F32, tag="b2")
            nc.vector.tensor_add(b2, t2, gnb_c)

            for ot in range(4):
                nc.gpsimd.tensor_scalar(
                    out=mmb[:, ot, :], in0=mmb[:, ot, :],
                    scalar1=a_col[:, ot : ot + 1], scalar2=b2[:, ot : ot + 1],
                    op0=OP.mult, op1=OP.add,
                )
                nc.sync.dma_start(out=outd[s, ot, :, :], in_=mmb[:, ot, :])

    nc.finalize()
    return nc


_NC_CACHE = {}


def _get_nc(use_f32r=True):
    if "nc" not in _NC_CACHE:
        _NC_CACHE["nc"] = build_nc()
    return _NC_CACHE["nc"]


def make_in_maps(inputs):
    f32 = np.float32
    x = np.ascontiguousarray(inputs["x"], dtype=f32).reshape(B, C, N)
    y = np.ascontiguousarray(inputs["y"], dtype=f32).reshape(B, DIMY)
    k_w = np.asarray(inputs["k_w"], f32)
    v_w = np.asarray(inputs["v_w"], f32)
    to_q_w = np.asarray(inputs["to_q_w"], f32)
    to_k_w = np.asarray(inputs["to_k_w"], f32)
    to_v_w = np.asarray(inputs["to_v_w"], f32)
    out_w = np.asarray(inputs["out_w"], f32)
    out_b = np.asarray(inputs["out_b"], f32)
    gn_g = np.asarray(inputs["gn_g"], f32)
    gn_b = np.asarray(inputs["gn_b"], f32)

    # host precompute: per-sample softmax-weighted value vector w[b,hd], and
    # the collapsed output weight W2[o,h] (all O(weights)/O(y) work)
    ky = y @ k_w.T                                   # [B, C]
    vy = y @ v_w.T
    rs_k = to_k_w.sum(1)                             # [C]
    rs_v = to_v_w.sum(1)
    ez = np.exp(rs_k[None, :, None] * ky[:, None, :])          # [B, hd, j]
    wvec = (ez * vy[:, None, :]).sum(-1) / ez.sum(-1)          # [B, C]
    W2 = SCALE * (
        out_w.reshape(C, HEADS, DHEAD) * rs_v.reshape(HEADS, DHEAD)[None]
    ).sum(-1)                                        # [C, 8]

    # reduction masks: [B, 4ot, 128p, 40]; col j: w if head==j, col 32+j: 1
    # (cols 8-31 zero-padded so num lands at psum partitions 0-7 and den at
    # 32-39 -- engine partition reads must be 32-aligned)
    hd = np.arange(C)
    head = hd // DHEAD
    ot_i, p_i = hd // 128, hd % 128
    wmask = np.zeros((B, 4, 128, 40), f32)
    wmask[:, ot_i, p_i, head] = wvec
    wmask[:, ot_i, p_i, 32 + head] = 1.0

    qwt = np.ascontiguousarray(
        to_q_w.T.reshape(4, 128, C).transpose(1, 0, 2)
    ).astype(BF16NP)                                 # [128p, 4ct, 512o]
    w2t = np.ascontiguousarray(W2.T.reshape(HEADS, 4, 128)).astype(BF16NP)
    gcols = np.ascontiguousarray(
        np.stack(
            [out_b.reshape(4, 128).T, gn_g.reshape(4, 128).T,
             gn_b.reshape(4, 128).T],
            axis=2,
        )
    ).astype(f32)                                    # [128, 4, 3]

    xb = x.reshape(B, 4, 128, N).astype(BF16NP)
    in_maps = []
    for core in range(NCORES):
        s0 = core * BPC
        m = {
            "xb": np.ascontiguousarray(xb[s0 : s0 + BPC]),
            "wm": np.ascontiguousarray(
                wmask[s0 : s0 + BPC].transpose(2, 0, 1, 3)
            ).astype(BF16NP),                        # [128, BPC, 4, 16]
            "qwt": qwt,
            "w2t": w2t,
            "gcols": gcols,
        }
        in_maps.append(m)
    return in_maps


def kernel(**inputs):
    nc = _get_nc()
    res = run_bass_kernel_spmd(nc, make_in_maps(inputs), list(range(NCORES)))
    out = np.concatenate([r["out"] for r in res.results], axis=0)  # [B,4,128,N]
    return out.reshape(B, C, N).astype(np.float32).reshape(B, C, 64, 64)


if __name__ == "__main__":
    rng = np.random.default_rng(0)
    inputs = {
        "x": rng.standard_normal((B, C, 64, 64), dtype=np.float32),
        "y": rng.standard_normal((B, 1, 1, DIMY), dtype=np.float32),
        "k_w": rng.standard_normal((C, DIMY), dtype=np.float32) * 0.02,
        "v_w": rng.standard_normal((C, DIMY), dtype=np.float32) * 0.02,
        "to_q_w": rng.standard_normal((C, C), dtype=np.float32) * 0.02,
        "to_k_w": rng.standard_normal((C, C), dtype=np.float32) * 0.02,
        "to_v_w": rng.standard_normal((C, C), dtype=np.float32) * 0.02,
        "out_w": rng.standard_normal((C, C), dtype=np.float32) * 0.02,
        "out_b": np.zeros(C, np.float32),
        "gn_g": np.ones(C, np.float32),
        "gn_b": np.zeros(C, np.float32),
    }
    out = kernel(**inputs)
    print("kernel ran, out shape", out.shape, "std", out.std())


# revision 20
# speedup vs baseline: 1.9804x; 1.2443x over previous
"""Trainium2 Bass kernel for nn_CrossAttention (16x512x64x64, 8 heads x 64).

Math (exact algebraic restructuring of the reference; see baseline notes):
  The tiled-k/v convs are rank-1, so everything except q = to_q_w @ x
  collapses to per-sample vectors computed on HOST from weights+y:
    w[b,hd]  = sum_j softmax_j(rs_k[hd]*ky[b,j]) * vy[b,j]
    W2[o,h]  = scale * sum_e out_w[o,64h+e] * rs_v[64h+e]
  Device computes, per sample:
    q[he,n]   = to_q_w @ x              (PE, bf16, [he,n] layout)
    e         = exp(q)                  (ACT)
    num/den   = mask-matmul over he     (PE: lhsT = [w-masked | 1-masked])
    s[h,n]    = num/den                 (DVE)
    mm[o,n]   = W2 @ s                  (PE, K=8)
    stats     = sum/sumsq of mm rows    (accum_out on the PSUM->SBUF copy + DVE)
    out       = A(o)*mm + B(o)          (GroupNorm affine, GPSIMD, in-place)

Sharding: data-parallel over batch, 2 samples/core, 8 cores, no collectives.
All I/O in bf16 (host converts); weights pre-transposed on host.
"""

import numpy as np
import ml_dtypes

import concourse.bass as bass
import concourse.mybir as mybir
import concourse.tile as tile
from concourse import bacc
from concourse.bass import ts
from concourse.bass_utils import run_bass_kernel_spmd

B, C, N = 16, 512, 4096
DIMY = 768
HEADS, DHEAD = 8, 64
NCORES = 8
BPC = B // NCORES
SCALE = DHEAD ** -0.5
EPS = 1e-5
F32 = mybir.dt.float32
BF16 = mybir.dt.bfloat16
AX = mybir.AxisListType.X
AF = mybir.ActivationFunctionType
OP = mybir.AluOpType
BF16NP = ml_dtypes.bfloat16


def build_nc(use_f32r=True):
    nc = bacc.Bacc()
    xd = nc.dram_tensor("xb", [BPC, 4, 128, N], BF16, kind="ExternalInput")
    qwtd = nc.dram_tensor("qwt", [128, 4, C], BF16, kind="ExternalInput")
    wmd = nc.dram_tensor("wm", [128, BPC, 4, 40], BF16, kind="ExternalInput")
    w2td = nc.dram_tensor("w2t", [HEADS, 4, 128], BF16, kind="ExternalInput")
    gcd = nc.dram_tensor("gcols", [128, 4, 3], F32, kind="ExternalInput")
    outd = nc.dram_tensor("out", [BPC, 4, 128, N], BF16, kind="ExternalOutput")

    from contextlib import ExitStack

    with tile.TileContext(nc) as tc, ExitStack() as ctx:
        persist = ctx.enter_context(tc.tile_pool(name="persist", bufs=1))
        xp = ctx.enter_context(tc.tile_pool(name="xp", bufs=8))
        ep = ctx.enter_context(tc.tile_pool(name="ep", bufs=6))
        sttp = ctx.enter_context(tc.tile_pool(name="sttp", bufs=4))
        rdp = ctx.enter_context(tc.tile_pool(name="rdp", bufs=3))
        mmp = ctx.enter_context(tc.tile_pool(name="mmp", bufs=2))
        accp = ctx.enter_context(tc.tile_pool(name="accp", bufs=4))
        scrp = ctx.enter_context(tc.tile_pool(name="scrp", bufs=2))
        smallp = ctx.enter_context(tc.tile_pool(name="smallp", bufs=10))
        rowp = ctx.enter_context(tc.tile_pool(name="rowp", bufs=8))
        psqp = ctx.enter_context(tc.tile_pool(name="psqp", bufs=3, space="PSUM"))
        ps16p = ctx.enter_context(tc.tile_pool(name="ps16p", bufs=2, space="PSUM"))
        psfp = ctx.enter_context(tc.tile_pool(name="psfp", bufs=3, space="PSUM"))

        # ---------------- weights / constants ----------------
        qwt = persist.tile([128, 4, C], BF16, tag="qwt")
        nc.sync.dma_start(out=qwt, in_=qwtd[:, :, :])
        wm = persist.tile([128, BPC, 4, 40], BF16, tag="wm")
        nc.sync.dma_start(out=wm, in_=wmd[:, :, :, :])
        w2t = persist.tile([HEADS, 4, 128], BF16, tag="w2t")
        nc.sync.dma_start(out=w2t, in_=w2td[:, :, :])
        gcols = persist.tile([128, 4, 3], F32, tag="gcols")
        nc.sync.dma_start(out=gcols, in_=gcd[:, :, :])
        outb_c = gcols[:, :, 0]
        gng_c = gcols[:, :, 1]
        gnb_c = gcols[:, :, 2]

        ones_col = persist.tile([128, 1], F32, tag="ones")
        nc.vector.memset(ones_col, 1.0)
        ones_row = persist.tile([1, 128], F32, tag="onesr")
        nc.vector.memset(ones_row, 1.0)
        zero_col = persist.tile([128, 1], F32, tag="zero")
        nc.vector.memset(zero_col, 0.0)
        nc.const_aps.aps[(F32, 0.0)] = zero_col[:, :]
        eps_col = persist.tile([128, 1], F32, tag="eps")
        nc.vector.memset(eps_col, EPS)
        nc.const_aps.aps[(F32, EPS)] = eps_col[:, :]

        # all x tiles up-front (sync HWDGE queue, 1 MB each)
        xs = {}
        for s in range(BPC):
            for ct in range(4):
                t = xp.tile([128, N], BF16, tag="xc")
                nc.sync.dma_start(out=t, in_=xd[s, ct, :, :])
                xs[(s, ct)] = t

        for s in range(BPC):
            mmb = mmp.tile([128, 4, N], BF16, tag="mmb")
            stats = accp.tile([128, 4, 8, 6], F32, tag="stats")

            for g in range(8):
                es = []
                for ot in range(4):
                    psq = psqp.tile([128, 512], F32, tag="psq")
                    for ct in range(4):
                        nc.tensor.matmul(
                            psq,
                            lhsT=qwt[:, ct, ts(ot, 128)],
                            rhs=xs[(s, ct)][:, ts(g, 512)],
                            start=(ct == 0),
                            stop=(ct == 3),
                        )
                    e = ep.tile([128, 512], BF16, tag="e")
                    nc.scalar.activation(out=e, in_=psq, func=AF.Exp)
                    es.append(e)

                ps16 = ps16p.tile([40, 512], F32, tag="ps16")
                for ot in range(4):
                    nc.tensor.matmul(
                        ps16,
                        lhsT=wm[:, s, ot, :],
                        rhs=es[ot],
                        start=(ot == 0),
                        stop=(ot == 3),
                    )
                den_sb = rdp.tile([8, 512], F32, tag="densb")
                nc.scalar.copy(out=den_sb, in_=ps16[32:40, :])
                rden = rdp.tile([8, 512], F32, tag="rden")
                nc.vector.reciprocal_approx_fast(out=rden, in_=den_sb)
                stt = sttp.tile([8, 512], BF16, tag="stt")
                nc.vector.tensor_mul(stt, ps16[0:8, :], rden)

                for ot in range(4):
                    psf = psfp.tile([128, 512], F32, tag="psf")
                    nc.tensor.matmul(
                        psf,
                        lhsT=w2t[:, ot, :],
                        rhs=stt,
                        start=True,
                        stop=True,
                    )
                    dst = mmb[:, ot, ts(g, 512)]
                    if (ot + g) % 2 == 0:
                        nc.scalar.copy(out=dst, in_=psf)
                    else:
                        nc.vector.tensor_copy(dst, psf)
                    nc.vector.bn_stats(out=stats[:, ot, g], in_=psf)

            # ---- GroupNorm(1) stats + affine ----
            mv8 = smallp.tile([128, 8], F32, tag="mv8")
            for ot in range(4):
                mv = smallp.tile([128, 2], F32, tag="mv")
                nc.vector.bn_aggr(out=mv, in_=stats[:, ot])
                m_ = mv8[:, ot : ot + 1]
                nc.vector.tensor_add(m_, mv[:, 0:1], outb_c[:, ot : ot + 1])
                msq = smallp.tile([128, 1], F32, tag="msq")
                nc.vector.tensor_mul(msq, m_, m_)
                nc.vector.tensor_add(mv8[:, 4 + ot : 5 + ot], mv[:, 1:2], msq)

            ps_tot = ps16p.tile([1, 8], F32, tag="ps16")
            nc.tensor.matmul(ps_tot, lhsT=ones_col, rhs=mv8, start=True, stop=True)
            rowt8 = rowp.tile([1, 8], F32, tag="rowt8")
            nc.scalar.copy(out=rowt8, in_=ps_tot)
            tt = rowp.tile([1, 2], F32, tag="tt")
            nc.vector.reduce_sum(
                out=tt, in_=rowt8.rearrange("p (a b) -> p a b", a=2), axis=AX
            )
            tt2 = rowp.tile([1, 2], F32, tag="tt2")
            nc.scalar.mul(out=tt2, in_=tt, mul=1.0 / C)  # {mu, E2}
            msq = rowp.tile([1, 1], F32, tag="msq")
            nc.vector.tensor_mul(msq, tt2[:, 0:1], tt2[:, 0:1])
            var = rowp.tile([1, 1], F32, tag="var")
            nc.vector.tensor_sub(var, tt2[:, 1:2], msq)
            sd = rowp.tile([1, 1], F32, tag="sd")
            nc.scalar.activation(out=sd, in_=var, func=AF.Sqrt, bias=EPS)
            rstd = rowp.tile([1, 1], F32, tag="rstd")
            nc.vector.reciprocal(rstd, sd)
            murow = rowp.tile([1, 2], F32, tag="murow")
            nc.vector.tensor_copy(murow[:, 0:1], tt2[:, 0:1])
            nc.vector.tensor_copy(murow[:, 1:2], rstd)
            ps_b = ps16p.tile([128, 2], F32, tag="ps16")
            nc.tensor.matmul(ps_b, lhsT=ones_row, rhs=murow, start=True, stop=True)
            msb = smallp.tile([128, 2], F32, tag="msb")
            nc.scalar.copy(out=msb, in_=ps_b)

            a_col = smallp.tile([128, 4], F32, tag="acol")
            nc.vector.tensor_scalar_mul(a_col, gng_c, msb[:, 1:2])
            t1 = smallp.tile([128, 4], F32, tag="t1")
            nc.vector.tensor_scalar(
                out=t1, in0=outb_c, scalar1=msb[:, 0:1], scalar2=None,
                op0=OP.subtract,
            )
            t2 = smallp.tile([128, 4], F32, tag="t2")
            nc.vector.tensor_mul(t2, a_col, t1)
            b2 = smallp.tile([128, 4], F32, tag="b2")
            nc.vector.tensor_add(b2, t2, gnb_c)

            for ot in range(4):
                eng = nc.gpsimd
                eng.tensor_scalar(
                    out=mmb[:, ot, :], in0=mmb[:, ot, :],
                    scalar1=a_col[:, ot : ot + 1], scalar2=b2[:, ot : ot + 1],
                    op0=OP.mult, op1=OP.add,
                )
                nc.sync.dma_start(out=outd[s, ot, :, :], in_=mmb[:, ot, :])

    nc.finalize()
    return nc


_NC_CACHE = {}


def _get_nc(use_f32r=True):
    if "nc" not in _NC_CACHE:
        _NC_CACHE["nc"] = build_nc()
    return _NC_CACHE["nc"]


def make_in_maps(inputs):
    f32 = np.float32
    x = np.ascontiguousarray(inputs["x"], dtype=f32).reshape(B, C, N)
    y = np.ascontiguousarray(inputs["y"], dtype=f32).reshape(B, DIMY)
    k_w = np.asarray(inputs["k_w"], f32)
    v_w = np.asarray(inputs["v_w"], f32)
    to_q_w = np.asarray(inputs["to_q_w"], f32)
    to_k_w = np.asarray(inputs["to_k_w"], f32)
    to_v_w = np.asarray(inputs["to_v_w"], f32)
    out_w = np.asarray(inputs["out_w"], f32)
    out_b = np.asarray(inputs["out_b"], f32)
    gn_g = np.asarray(inputs["gn_g"], f32)
    gn_b = np.asarray(inputs["gn_b"], f32)

    # host precompute: per-sample softmax-weighted value vector w[b,hd], and
    # the collapsed output weight W2[o,h] (all O(weights)/O(y) work)
    ky = y @ k_w.T                                   # [B, C]
    vy = y @ v_w.T
    rs_k = to_k_w.sum(1)                             # [C]
    rs_v = to_v_w.sum(1)
    ez = np.exp(rs_k[None, :, None] * ky[:, None, :])          # [B, hd, j]
    wvec = (ez * vy[:, None, :]).sum(-1) / ez.sum(-1)          # [B, C]
    W2 = SCALE * (
        out_w.reshape(C, HEADS, DHEAD) * rs_v.reshape(HEADS, DHEAD)[None]
    ).sum(-1)                                        # [C, 8]

    # reduction masks: [B, 4ot, 128p, 40]; col j: w if head==j, col 32+j: 1
    # (cols 8-31 zero-padded so num lands at psum partitions 0-7 and den at
    # 32-39 -- engine partition reads must be 32-aligned)
    hd = np.arange(C)
    head = hd // DHEAD
    ot_i, p_i = hd // 128, hd % 128
    wmask = np.zeros((B, 4, 128, 40), f32)
    wmask[:, ot_i, p_i, head] = wvec
    wmask[:, ot_i, p_i, 32 + head] = 1.0

    qwt = np.ascontiguousarray(
        to_q_w.T.reshape(4, 128, C).transpose(1, 0, 2)
    ).astype(BF16NP)                                 # [128p, 4ct, 512o]
    w2t = np.ascontiguousarray(W2.T.reshape(HEADS, 4, 128)).astype(BF16NP)
    gcols = np.ascontiguousarray(
        np.stack(
            [out_b.reshape(4, 128).T, gn_g.reshape(4, 128).T,
             gn_b.reshape(4, 128).T],
            axis=2,
        )
    ).astype(f32)                                    # [128, 4, 3]

    xb = x.reshape(B, 4, 128, N).astype(BF16NP)
    in_maps = []
    for core in range(NCORES):
        s0 = core * BPC
        m = {
            "xb": np.ascontiguousarray(xb[s0 : s0 + BPC]),
            "wm": np.ascontiguousarray(
                wmask[s0 : s0 + BPC].transpose(2, 0, 1, 3)
            ).astype(BF16NP),                        # [128, BPC, 4, 16]
            "qwt": qwt,
            "w2t": w2t,
            "gcols": gcols,
        }
        in_maps.append(m)
    return in_maps


def kernel(**inputs):
    nc = _get_nc()
    res = run_bass_kernel_spmd(nc, make_in_maps(inputs), list(range(NCORES)))
    out = np.concatenate([r["out"] for r in res.results], axis=0)  # [B,4,128,N]
    return out.reshape(B, C, N).astype(np.float32).reshape(B, C, 64, 64)


if __name__ == "__main__":
    rng = np.random.default_rng(0)
    inputs = {
        "x": rng.standard_normal((B, C, 64, 64), dtype=np.float32),
        "y": rng.standard_normal((B, 1, 1, DIMY), dtype=np.float32),
        "k_w": rng.standard_normal((C, DIMY), dtype=np.float32) * 0.02,
        "v_w": rng.standard_normal((C, DIMY), dtype=np.float32) * 0.02,
        "to_q_w": rng.standard_normal((C, C), dtype=np.float32) * 0.02,
        "to_k_w": rng.standard_normal((C, C), dtype=np.float32) * 0.02,
        "to_v_w": rng.standard_normal((C, C), dtype=np.float32) * 0.02,
        "out_w": rng.standard_normal((C, C), dtype=np.float32) * 0.02,
        "out_b": np.zeros(C, np.float32),
        "gn_g": np.ones(C, np.float32),
        "gn_b": np.zeros(C, np.float32),
    }
    out = kernel(**inputs)
    print("kernel ran, out shape", out.shape, "std", out.std())


# revision 23
# speedup vs baseline: 2.0756x; 1.0481x over previous
"""Trainium2 Bass kernel for nn_CrossAttention (16x512x64x64, 8 heads x 64).

Math (exact algebraic restructuring of the reference; see baseline notes):
  The tiled-k/v convs are rank-1, so everything except q = to_q_w @ x
  collapses to per-sample vectors computed on HOST from weights+y:
    w[b,hd]  = sum_j softmax_j(rs_k[hd]*ky[b,j]) * vy[b,j]
    W2[o,h]  = scale * sum_e out_w[o,64h+e] * rs_v[64h+e]
  Device computes, per sample:
    q[he,n]   = to_q_w @ x              (PE, bf16, [he,n] layout)
    e         = exp(q)                  (ACT)
    num/den   = mask-matmul over he     (PE: lhsT = [w-masked | 1-masked])
    s[h,n]    = num/den                 (DVE)
    mm[o,n]   = W2 @ s                  (PE, K=8)
    stats     = sum/sumsq of mm rows    (accum_out on the PSUM->SBUF copy + DVE)
    out       = A(o)*mm + B(o)          (GroupNorm affine, GPSIMD, in-place)

Sharding: data-parallel over batch, 2 samples/core, 8 cores, no collectives.
All I/O in bf16 (host converts); weights pre-transposed on host.
"""

import numpy as np
import ml_dtypes

import concourse.bass as bass
import concourse.mybir as mybir
import concourse.tile as tile
from concourse import bacc
from concourse.bass import ts
from concourse.bass_utils import run_bass_kernel_spmd

B, C, N = 16, 512, 4096
DIMY = 768
HEADS, DHEAD = 8, 64
NCORES = 8
BPC = B // NCORES
SCALE = DHEAD ** -0.5
EPS = 1e-5
F32 = mybir.dt.float32
BF16 = mybir.dt.bfloat16
AX = mybir.AxisListType.X
AF = mybir.ActivationFunctionType
OP = mybir.AluOpType
BF16NP = ml_dtypes.bfloat16


def build_nc(use_f32r=True):
    nc = bacc.Bacc()
    xd = nc.dram_tensor("xb", [BPC, 4, 128, N], BF16, kind="ExternalInput")
    qwtd = nc.dram_tensor("qwt", [128, 4, C], BF16, kind="ExternalInput")
    wmd = nc.dram_tensor("wm", [128, BPC, 4, 40], BF16, kind="ExternalInput")
    w2td = nc.dram_tensor("w2t", [HEADS, 4, 128], BF16, kind="ExternalInput")
    gcd = nc.dram_tensor("gcols", [128, 4, 3], F32, kind="ExternalInput")
    outd = nc.dram_tensor("out", [BPC, 4, 128, N], BF16, kind="ExternalOutput")

    from contextlib import ExitStack

    with tile.TileContext(nc) as tc, ExitStack() as ctx:
        persist = ctx.enter_context(tc.tile_pool(name="persist", bufs=1))
        xp = ctx.enter_context(tc.tile_pool(name="xp", bufs=8))
        ep = ctx.enter_context(tc.tile_pool(name="ep", bufs=10))
        sttp = ctx.enter_context(tc.tile_pool(name="sttp", bufs=4))
        rdp = ctx.enter_context(tc.tile_pool(name="rdp", bufs=4))
        mmp = ctx.enter_context(tc.tile_pool(name="mmp", bufs=2))
        accp = ctx.enter_context(tc.tile_pool(name="accp", bufs=4))
        stgp = ctx.enter_context(tc.tile_pool(name="stgp", bufs=4))
        smallp = ctx.enter_context(tc.tile_pool(name="smallp", bufs=10))
        rowp = ctx.enter_context(tc.tile_pool(name="rowp", bufs=8))
        psqp = ctx.enter_context(tc.tile_pool(name="psqp", bufs=3, space="PSUM"))
        ps16p = ctx.enter_context(tc.tile_pool(name="ps16p", bufs=2, space="PSUM"))
        psfp = ctx.enter_context(tc.tile_pool(name="psfp", bufs=3, space="PSUM"))

        # ---------------- weights / constants ----------------
        qwt = persist.tile([128, 4, C], BF16, tag="qwt")
        nc.sync.dma_start(out=qwt, in_=qwtd[:, :, :])
        wm = persist.tile([128, BPC, 4, 40], BF16, tag="wm")
        nc.sync.dma_start(out=wm, in_=wmd[:, :, :, :])
        w2t = persist.tile([HEADS, 4, 128], BF16, tag="w2t")
        nc.sync.dma_start(out=w2t, in_=w2td[:, :, :])
        gcols = persist.tile([128, 4, 3], F32, tag="gcols")
        nc.sync.dma_start(out=gcols, in_=gcd[:, :, :])
        outb_c = gcols[:, :, 0]
        gng_c = gcols[:, :, 1]
        gnb_c = gcols[:, :, 2]

        ones_col = persist.tile([128, 1], F32, tag="ones")
        nc.vector.memset(ones_col, 1.0)
        ones_row = persist.tile([1, 128], F32, tag="onesr")
        nc.vector.memset(ones_row, 1.0)
        zero_col = persist.tile([128, 1], F32, tag="zero")
        nc.vector.memset(zero_col, 0.0)
        nc.const_aps.aps[(F32, 0.0)] = zero_col[:, :]
        eps_col = persist.tile([128, 1], F32, tag="eps")
        nc.vector.memset(eps_col, EPS)
        nc.const_aps.aps[(F32, EPS)] = eps_col[:, :]

        # x tiles: sample 0 in halves (faster lead-in), rest whole
        xs = {}
        for s in range(BPC):
            for ct in range(4):
                xc = xp.tile([128, N], BF16, tag="xc")
                xs[(s, ct)] = xc
        for ct in range(4):
            nc.sync.dma_start(out=xs[(0, ct)][:, 0:2048], in_=xd[0, ct, :, 0:2048])
        for ct in range(4):
            nc.sync.dma_start(out=xs[(0, ct)][:, 2048:N], in_=xd[0, ct, :, 2048:N])
        for s in range(1, BPC):
            for ct in range(4):
                nc.sync.dma_start(out=xs[(s, ct)], in_=xd[s, ct, :, :])

        mmbs, statss = {}, {}

        def emit_q(s, g):
            es = []
            for ot in range(4):
                psq = psqp.tile([128, 512], F32, tag="psq")
                for ct in range(4):
                    nc.tensor.matmul(
                        psq,
                        lhsT=qwt[:, ct, ts(ot, 128)],
                        rhs=xs[(s, ct)][:, ts(g, 512)],
                        start=(ct == 0),
                        stop=(ct == 3),
                    )
                e = ep.tile([128, 512], BF16, tag="e")
                nc.scalar.activation(out=e, in_=psq, func=AF.Exp)
                es.append(e)
            return es

        def emit_redfin(s, g, es):
            ps16 = ps16p.tile([40, 512], F32, tag="ps16")
            for ot in range(4):
                nc.tensor.matmul(
                    ps16,
                    lhsT=wm[:, s, ot, :],
                    rhs=es[ot],
                    start=(ot == 0),
                    stop=(ot == 3),
                )
            den_sb = rdp.tile([8, 512], F32, tag="densb")
            nc.scalar.copy(out=den_sb, in_=ps16[32:40, :])
            rden = rdp.tile([8, 512], F32, tag="rden")
            nc.vector.reciprocal_approx_fast(out=rden, in_=den_sb)
            stt = sttp.tile([8, 512], BF16, tag="stt")
            nc.vector.tensor_mul(stt, ps16[0:8, :], rden)
            mmb, stats = mmbs[s], statss[s]
            for ot in range(4):
                psf = psfp.tile([128, 512], F32, tag="psf")
                nc.tensor.matmul(
                    psf, lhsT=w2t[:, ot, :], rhs=stt, start=True, stop=True
                )
                dst = mmb[:, ot, ts(g, 512)]
                if (ot + g) % 2 == 0:
                    nc.scalar.copy(out=dst, in_=psf)
                else:
                    nc.vector.tensor_copy(dst, psf)
                nc.vector.bn_stats(out=stats[:, ot, g], in_=psf)

        def emit_epilogue(s):
            mmb, stats = mmbs[s], statss[s]
            mv8 = smallp.tile([128, 8], F32, tag="mv8")
            for ot in range(4):
                mv = smallp.tile([128, 2], F32, tag="mv")
                nc.vector.bn_aggr(out=mv, in_=stats[:, ot])
                m_ = mv8[:, ot : ot + 1]
                nc.vector.tensor_add(m_, mv[:, 0:1], outb_c[:, ot : ot + 1])
                msq = smallp.tile([128, 1], F32, tag="msq")
                nc.vector.tensor_mul(msq, m_, m_)
                nc.vector.tensor_add(mv8[:, 4 + ot : 5 + ot], mv[:, 1:2], msq)

            ps_tot = ps16p.tile([1, 8], F32, tag="ps16")
            nc.tensor.matmul(ps_tot, lhsT=ones_col, rhs=mv8, start=True, stop=True)
            rowt8 = rowp.tile([1, 8], F32, tag="rowt8")
            nc.scalar.copy(out=rowt8, in_=ps_tot)
            tt = rowp.tile([1, 2], F32, tag="tt")
            nc.vector.reduce_sum(
                out=tt, in_=rowt8.rearrange("p (a b) -> p a b", a=2), axis=AX
            )
            tt2 = rowp.tile([1, 2], F32, tag="tt2")
            nc.scalar.mul(out=tt2, in_=tt, mul=1.0 / C)  # {mu, E2}
            msq = rowp.tile([1, 1], F32, tag="msq")
            nc.vector.tensor_mul(msq, tt2[:, 0:1], tt2[:, 0:1])
            var = rowp.tile([1, 1], F32, tag="var")
            nc.vector.tensor_sub(var, tt2[:, 1:2], msq)
            sd = rowp.tile([1, 1], F32, tag="sd")
            nc.scalar.activation(out=sd, in_=var, func=AF.Sqrt, bias=EPS)
            rstd = rowp.tile([1, 1], F32, tag="rstd")
            nc.vector.reciprocal(rstd, sd)
            murow = rowp.tile([1, 2], F32, tag="murow")
            nc.vector.tensor_copy(murow[:, 0:1], tt2[:, 0:1])
            nc.vector.tensor_copy(murow[:, 1:2], rstd)
            ps_b = ps16p.tile([128, 2], F32, tag="ps16")
            nc.tensor.matmul(ps_b, lhsT=ones_row, rhs=murow, start=True, stop=True)
            msb = smallp.tile([128, 2], F32, tag="msb")
            nc.scalar.copy(out=msb, in_=ps_b)

            a_col = smallp.tile([128, 4], F32, tag="acol")
            nc.vector.tensor_scalar_mul(a_col, gng_c, msb[:, 1:2])
            t1 = smallp.tile([128, 4], F32, tag="t1")
            nc.vector.tensor_scalar(
                out=t1, in0=outb_c, scalar1=msb[:, 0:1], scalar2=None,
                op0=OP.subtract,
            )
            t2 = smallp.tile([128, 4], F32, tag="t2")
            nc.vector.tensor_mul(t2, a_col, t1)
            b2 = smallp.tile([128, 4], F32, tag="b2")
            nc.vector.tensor_add(b2, t2, gnb_c)

            for ot in range(4):
                stg = stgp.tile([128, N], BF16, tag="stg")
                nc.vector.tensor_scalar(
                    out=stg, in0=mmb[:, ot, :],
                    scalar1=a_col[:, ot : ot + 1], scalar2=b2[:, ot : ot + 1],
                    op0=OP.mult, op1=OP.add,
                )
                nc.sync.dma_start(out=outd[s, ot, :, :], in_=stg)

        # software pipeline: PE runs q(g+1) while ACT/DVE produce e/stt for g
        pend = None
        for s in range(BPC):
            for g in range(8):
                if g == 0:
                    mmb_t = mmp.tile([128, 4, N], BF16, tag="mmb")
                    stats_t = accp.tile([128, 4, 8, 6], F32, tag="stats")
                    mmbs[s] = mmb_t
                    statss[s] = stats_t
                es = emit_q(s, g)
                if pend is not None:
                    ps_, pg_, pes_ = pend
                    emit_redfin(ps_, pg_, pes_)
                    if pg_ == 7:
                        emit_epilogue(ps_)
                pend = (s, g, es)
        ps_, pg_, pes_ = pend
        emit_redfin(ps_, pg_, pes_)
        emit_epilogue(ps_)

    nc.finalize()
    return nc


_NC_CACHE = {}


def _get_nc(use_f32r=True):
    if "nc" not in _NC_CACHE:
        _NC_CACHE["nc"] = build_nc()
    return _NC_CACHE["nc"]


def make_in_maps(inputs):
    f32 = np.float32
    x = np.ascontiguousarray(inputs["x"], dtype=f32).reshape(B, C, N)
    y = np.ascontiguousarray(inputs["y"], dtype=f32).reshape(B, DIMY)
    k_w = np.asarray(inputs["k_w"], f32)
    v_w = np.asarray(inputs["v_w"], f32)
    to_q_w = np.asarray(inputs["to_q_w"], f32)
    to_k_w = np.asarray(inputs["to_k_w"], f32)
    to_v_w = np.asarray(inputs["to_v_w"], f32)
    out_w = np.asarray(inputs["out_w"], f32)
    out_b = np.asarray(inputs["out_b"], f32)
    gn_g = np.asarray(inputs["gn_g"], f32)
    gn_b = np.asarray(inputs["gn_b"], f32)

    # host precompute: per-sample softmax-weighted value vector w[b,hd], and
    # the collapsed output weight W2[o,h] (all O(weights)/O(y) work)
    ky = y @ k_w.T                                   # [B, C]
    vy = y @ v_w.T
    rs_k = to_k_w.sum(1)                             # [C]
    rs_v = to_v_w.sum(1)
    ez = np.exp(rs_k[None, :, None] * ky[:, None, :])          # [B, hd, j]
    wvec = (ez * vy[:, None, :]).sum(-1) / ez.sum(-1)          # [B, C]
    W2 = SCALE * (
        out_w.reshape(C, HEADS, DHEAD) * rs_v.reshape(HEADS, DHEAD)[None]
    ).sum(-1)                                        # [C, 8]

    # reduction masks: [B, 4ot, 128p, 40]; col j: w if head==j, col 32+j: 1
    # (cols 8-31 zero-padded so num lands at psum partitions 0-7 and den at
    # 32-39 -- engine partition reads must be 32-aligned)
    hd = np.arange(C)
    head = hd // DHEAD
    ot_i, p_i = hd // 128, hd % 128
    wmask = np.zeros((B, 4, 128, 40), f32)
    wmask[:, ot_i, p_i, head] = wvec
    wmask[:, ot_i, p_i, 32 + head] = 1.0

    qwt = np.ascontiguousarray(
        to_q_w.T.reshape(4, 128, C).transpose(1, 0, 2)
    ).astype(BF16NP)                                 # [128p, 4ct, 512o]
    w2t = np.ascontiguousarray(W2.T.reshape(HEADS, 4, 128)).astype(BF16NP)
    gcols = np.ascontiguousarray(
        np.stack(
            [out_b.reshape(4, 128).T, gn_g.reshape(4, 128).T,
             gn_b.reshape(4, 128).T],
            axis=2,
        )
    ).astype(f32)                                    # [128, 4, 3]

    xb = x.reshape(B, 4, 128, N).astype(BF16NP)
    in_maps = []
    for core in range(NCORES):
        s0 = core * BPC
        m = {
            "xb": np.ascontiguousarray(xb[s0 : s0 + BPC]),
            "wm": np.ascontiguousarray(
                wmask[s0 : s0 + BPC].transpose(2, 0, 1, 3)
            ).astype(BF16NP),                        # [128, BPC, 4, 16]
            "qwt": qwt,
            "w2t": w2t,
            "gcols": gcols,
        }
        in_maps.append(m)
    return in_maps


def kernel(**inputs):
    nc = _get_nc()
    res = run_bass_kernel_spmd(nc, make_in_maps(inputs), list(range(NCORES)))
    out = np.concatenate([r["out"] for r in res.results], axis=0)  # [B,4,128,N]
    return out.reshape(B, C, N).astype(np.float32).reshape(B, C, 64, 64)


if __name__ == "__main__":
    rng = np.random.default_rng(0)
    inputs = {
        "x": rng.standard_normal((B, C, 64, 64), dtype=np.float32),
        "y": rng.standard_normal((B, 1, 1, DIMY), dtype=np.float32),
        "k_w": rng.standard_normal((C, DIMY), dtype=np.float32) * 0.02,
        "v_w": rng.standard_normal((C, DIMY), dtype=np.float32) * 0.02,
        "to_q_w": rng.standard_normal((C, C), dtype=np.float32) * 0.02,
        "to_k_w": rng.standard_normal((C, C), dtype=np.float32) * 0.02,
        "to_v_w": rng.standard_normal((C, C), dtype=np.float32) * 0.02,
        "out_w": rng.standard_normal((C, C), dtype=np.float32) * 0.02,
        "out_b": np.zeros(C, np.float32),
        "gn_g": np.ones(C, np.float32),
        "gn_b": np.zeros(C, np.float32),
    }
    out = kernel(**inputs)
    print("kernel ran, out shape", out.shape, "std", out.std())
